# revision 23
# baseline (speedup 1.0000x reference)
"""MHA kernel for Trainium2, 8-core tensor-parallel (2 heads per core).

Problem (hardcoded): x [2, 2048, 1024] fp32, Wq/Wk/Wv/Wo [1024, 1024],
bq/bk/bv/bo [1024], H=16 heads, DH=64.  out = MHA(x).

Sharding: heads are split 8 ways (2 heads = 128 proj columns per core).
Each core computes its heads' attention output and a partial output
projection (row-parallel Wo); the host sums the 8 partials and adds the
closed-form bias terms (bv @ Wo + bo).

v2 design (ScalarE-exp is the bottleneck engine at ~147us; everything
else is scheduled to hide under it):
  - scores S^T [128 k, 2h x 512 q] per ktile (two heads concurrently on
    PE row groups 0-63 / 64-127), exp on ScalarE per ktile (N=1024).
  - AV swapped: pt tile [128 k, 128 q] is the STATIONARY operand (full
    128-col array, FWL-eligible), V augmented with a ones column is the
    MOVING operand (N=65) -> out [128 q, 64 dh | denom] token-major with
    the softmax denominator accumulated for free in column 64.
  - normalize = DVE tensor_scalar divide by the per-partition denom.
  - transpose back to dh-major via one [128,128] identity matmul per
    q-subtile (both heads at once), then the usual row-parallel outproj.
  - QKV/V projections are interleaved into the attention phase's PE idle
    slots (PE has ~9us/combo spare under the 18.4us/combo exp pace).
  - partial outputs DMA'd out in bf16; host accumulates in fp32.
"""

import numpy as np
import ml_dtypes

D = 1024
T = 4096          # B*S tokens
S = 2048
B = 2
NH = 2            # heads per core
DH = 64
NCORES = 8
SCALE = 0.125     # 1/sqrt(DH)
NKT = S // 128    # 16 key tiles per batch
NQC = S // 512    # 4 query chunks per batch
NCK = T // 512    # 8 x^T column chunks
VSLOT = DH + 1    # 65: V columns + ones column

_CACHE = {}


def _build_nc(reps=1):
    import concourse.bacc as bacc
    import concourse.mybir as mybir
    import concourse.tile as tile
    from concourse.hw_specs import get_activation_tables as _gat

    # Pin Exp and Ln to the one table set that holds both, so the
    # table-load placement pass emits a single ACT_TABLE_LOAD instead of
    # thrashing between exp_and_others and natural_log every combo.
    def _pinned_tables(arch):
        out = {}
        for k, fns in _gat(arch).items():
            if k != "natural_log_exp_and_others":
                fns = {f for f in fns if f.name not in ("Exp", "Ln")}
            out[k] = fns
        return out
    bacc.get_activation_tables = _pinned_tables

    dt = mybir.dt
    f32, bf16, i32 = dt.float32, dt.bfloat16, dt.int32

    nc = bacc.Bacc("TRN2", target_bir_lowering=False, debug=False,
                   num_devices=NCORES)

    xT = nc.dram_tensor("xT", [D, T], bf16, kind="ExternalInput")
    wq_d = nc.dram_tensor("wq", [D, 128], bf16, kind="ExternalInput")
    wk_d = nc.dram_tensor("wk", [D, 128], bf16, kind="ExternalInput")
    wv_d = nc.dram_tensor("wv", [D, 128], bf16, kind="ExternalInput")
    wo_d = nc.dram_tensor("wo", [128, D], bf16, kind="ExternalInput")
    bq_d = nc.dram_tensor("bq", [128, 1], f32, kind="ExternalInput")
    bk_d = nc.dram_tensor("bk", [128, 1], f32, kind="ExternalInput")
    outp = nc.dram_tensor("outp", [T, D], bf16, kind="ExternalOutput")

    with tile.TileContext(nc) as tc:
      for _rep in range(reps):
        with (
            tc.tile_pool(name="persist", bufs=1) as pp,
            tc.tile_pool(name="pt", bufs=2) as ptp,
            tc.tile_pool(name="ot", bufs=3) as otp,       # onorm_tok [128,128]
            tc.tile_pool(name="otT", bufs=4) as otTp,     # onormT [128,128]
            tc.tile_pool(name="dn", bufs=2) as dnp,       # denom sbuf [128,8]
            tc.tile_pool(name="outsb", bufs=3) as osp,
            tc.tile_pool(name="st_ps", bufs=2, space="PSUM") as stp,   # 4 banks
            tc.tile_pool(name="av_ps", bufs=1, space="PSUM") as avp,   # 2 banks
            tc.tile_pool(name="mm_ps", bufs=2, space="PSUM") as mmp,   # 2 banks
        ):
            # ---- constants / weights / x^T (one DMA per 512-col chunk,
            # ordered so K(b0,c0) can start as early as possible) ----
            wq = pp.tile([128, D], bf16, tag="wq")
            wk = pp.tile([128, D], bf16, tag="wk")
            wv = pp.tile([128, D], bf16, tag="wv")
            wo = pp.tile([128, D], bf16, tag="wo")
            bq = pp.tile([128, 1], f32, tag="bq")
            bk = pp.tile([128, 1], f32, tag="bk")
            xt_all = pp.tile([128, 8 * T], bf16, tag="xt")
            xt4 = xt_all.rearrange("p (d t) -> p d t", d=8)
            xsrc = xT.ap().rearrange("(d p) c -> p d c", p=128)

            def dma_w(w_sb, w_dr):
                nc.sync.dma_start(
                    out=w_sb.rearrange("p (t c) -> p t c", c=128),
                    in_=w_dr.ap().rearrange("(t p) c -> p t c", p=128))

            def dma_chunk(nck):
                cs = slice(nck * 512, (nck + 1) * 512)
                nc.sync.dma_start(out=xt4[:, :, cs], in_=xsrc[:, :, cs])

            dma_chunk(0)
            dma_w(wk, wk_d)
            nc.sync.dma_start(out=bk[:, :], in_=bk_d.ap()[:, :])
            dma_w(wq, wq_d)
            nc.sync.dma_start(out=bq[:, :], in_=bq_d.ap()[:, :])
            dma_chunk(1)
            dma_w(wv, wv_d)
            nc.sync.dma_start(out=wo[:, :], in_=wo_d.ap()[:, :])
            for nck in range(2, NCK):
                dma_chunk(nck)
            ones1 = pp.tile([128, 1], f32, tag="ones1")
            nc.vector.memset(ones1[:, :], 1.0)

            # identity [128,128] bf16 for PE transposes: (f - p) == 0
            idx = pp.tile([128, 128], i32, tag="idx")
            nc.gpsimd.iota(idx[:, :], pattern=[[1, 128]], base=0,
                           channel_multiplier=-1)
            ident = pp.tile([128, 128], bf16, tag="ident")
            nc.vector.tensor_scalar(ident[:, :], idx[:, :], 0, None,
                                    op0=mybir.AluOpType.is_equal)

            # ---- persistent proj outputs ----
            qt = pp.tile([128, T], bf16, tag="qt")
            kt = pp.tile([128, T], bf16, tag="kt")
            vtm = []
            for b in range(B):
                v_sb = pp.tile([128, NH * NKT * VSLOT], bf16, tag=f"v{b}")
                v4 = v_sb.rearrange("p (h k c) -> p h k c", h=NH, k=NKT)
                nc.vector.memset(v4[:, :, :, DH:DH + 1], 1.0)
                vtm.append(v_sb)

            wq3 = wq.rearrange("p (t c) -> p t c", c=128)
            wk3 = wk.rearrange("p (t c) -> p t c", c=128)
            wv3 = wv.rearrange("p (t c) -> p t c", c=128)

            # single persistent AV accumulator (re-zeroed by each combo's
            # deferred norm chain, so the memset always follows the reads)
            av = avp.tile([128, NH * 4 * VSLOT], f32, tag="av")
            av4 = av.rearrange("p (h s c) -> p h s c", h=NH, s=4)
            nc.vector.memset(av[:, :], 0.0)

            # ---- filler work units (each: 8 chained MMs + 1 DVE evac) ----
            def emit_QU(b, qc):
                cs = slice(b * S + qc * 512, b * S + qc * 512 + 512)
                ps = mmp.tile([128, 512], f32, tag="mm",
                              name=f"qproj{b}_{qc}")
                for d in range(8):
                    nc.tensor.matmul(ps[:, :], wq3[:, d, :], xt4[:, d, cs],
                                     start=(d == 0), stop=(d == 7))
                nc.vector.tensor_scalar_add(qt[:, cs], ps[:, :], bq[:, :])

            def emit_KU(b, c, c0=0, c1=512):
                cs = slice(b * S + c * 512 + c0, b * S + c * 512 + c1)
                ps = mmp.tile([128, 512], f32, tag="mm",
                              name=f"kproj{b}_{c}_{c0}")
                for d in range(8):
                    nc.tensor.matmul(ps[:, 0:c1 - c0], wk3[:, d, :],
                                     xt4[:, d, cs],
                                     start=(d == 0), stop=(d == 7))
                nc.vector.tensor_scalar_add(kt[:, cs], ps[:, 0:c1 - c0],
                                            bk[:, :])

            def emit_VU(b, k):
                tok0 = b * S + k * 128
                v4 = vtm[b].rearrange("p (h k c) -> p h k c", h=NH, k=NKT)
                ps = mmp.tile([128, 128], f32, tag="mm",
                              name=f"vproj{b}_{k}")
                for d in range(8):
                    nc.tensor.matmul(ps[:, :], xt4[:, d, tok0:tok0 + 128],
                                     wv3[:, d, :],
                                     start=(d == 0), stop=(d == 7))
                nc.vector.tensor_copy(
                    v4[:, :, k, 0:DH],
                    ps.rearrange("p (h c) -> p h c", h=NH)[:, :, :])

            # ---- per-combo state ----
            combo_ot = {}    # m -> list of 4 onorm_tok tiles

            def emit_trans_out(m):
                # transpose (both heads at once) + outproj + out DMA for
                # combo m (emitted one combo later)
                b, qc = divmod(m, NQC)
                q0 = b * S + qc * 512
                ots = combo_ot.pop(m)
                oTs = []
                for s4 in range(4):
                    tp = mmp.tile([128, 128], f32, tag="mm",
                                  name=f"tp{m}_{s4}")
                    nc.tensor.matmul(tp[:, :], ots[s4][:, :], ident[:, :],
                                     start=True, stop=True)
                    oT = otTp.tile([128, 128], bf16, tag="otT",
                                   name=f"otT{m}_{s4}")
                    nc.vector.tensor_copy(oT[:, :], tp[:, :])
                    oTs.append(oT)
                for s4 in range(4):
                    oT = oTs[s4]
                    for jc in range(2):
                        op = mmp.tile([128, 512], f32, tag="mm",
                                      name=f"op{m}_{s4}_{jc}")
                        nc.tensor.matmul(
                            op[:, :], oT[:, :],
                            wo[:, jc * 512:(jc + 1) * 512],
                            start=True, stop=True)
                        osb = osp.tile([128, 512], bf16, tag="outsb",
                                       name=f"osb{m}_{s4}_{jc}")
                        nc.vector.tensor_copy(osb[:, :], op[:, :])
                        r0 = q0 + s4 * 128
                        nc.sync.dma_start(
                            out=outp.ap()[r0:r0 + 128,
                                          jc * 512:(jc + 1) * 512],
                            in_=osb[:, :])

            # ---- combo emission ----
            def emit_combo(m, pre, quad_fillers, vu_list=(), norm_prev=None):
                """pre: fillers before scores; quad_fillers: list of 4
                filler-lists emitted after each 4-ktile score block; vu_b:
                V-proj units (b, kt) to spread one-per-ktile;
                norm_prev: the previous combo's deferred norm closure
                (emitted after ktile 1's exp so its Ln/Exp never stall the
                scalar queue).  AV matmuls for ktile k are interleaved
                right after the scores of ktile k+2 (exp(k) is done by
                then thanks to the st double-buffer pacing), so the PE
                never piles a serial AV block between combos."""
                b, qc = divmod(m, NQC)
                q0 = b * S + qc * 512
                v4 = vtm[b].rearrange("p (h k c) -> p h k c", h=NH, k=NKT)
                for f in pre:
                    f()
                pt = ptp.tile([128, NH * NKT * 512], bf16, tag="pt",
                              name=f"pt{m}")
                pt3 = pt.rearrange("p (h k q) -> p h k q", h=NH, k=NKT)

                # AV swapped: pt stationary [128k,128q], V|1 moving (N=65).
                # The 8 (h,s4) accumulation groups interleave within shared
                # PSUM banks, so hardware zero-on-start (2KB region
                # granularity) would wipe neighbors: the tile is memset
                # (by the preamble / previous norm chain) and accumulated
                # with start=False throughout.
                def emit_av(kti):
                    for h in range(NH):
                        for s4 in range(4):
                            nc.tensor.matmul(
                                av4[:, h, s4, :],
                                pt3[:, h, kti, s4 * 128:(s4 + 1) * 128],
                                v4[:, h, kti, :],
                                start=False, stop=(kti == NKT - 1),
                                skip_group_check=True)

                for kti in range(NKT):
                    k0 = b * S + kti * 128
                    st = stp.tile([128, 1024], f32, tag="st",
                                  name=f"st{m}_{kti}")
                    for h in range(NH):
                        hp = h * DH
                        nc.tensor.matmul(
                            st[:, h * 512:(h + 1) * 512],
                            kt[hp:hp + DH, k0:k0 + 128],
                            qt[hp:hp + DH, q0:q0 + 512],
                            start=True, stop=True)
                    nc.scalar.activation(
                        pt3[:, :, kti, :], st[:, :],
                        mybir.ActivationFunctionType.Exp, scale=SCALE)
                    if kti == 1 and norm_prev is not None:
                        norm_prev()
                    if kti < len(vu_list):
                        emit_VU(*vu_list[kti])
                    if kti >= 2:
                        emit_av(kti - 2)
                    if kti % 4 == 3:
                        for f in quad_fillers[kti // 4]:
                            f()
                emit_av(NKT - 2)
                emit_av(NKT - 1)

                def norm():
                    # denoms -> SBUF, recip via Ln/Exp on ScalarE, DVE
                    # mult, then re-zero the shared AV accumulator for the
                    # next combo (after all reads).
                    dn = dnp.tile([128, 2 * NH * 4], f32, tag="dn",
                                  name=f"dn{m}")
                    dn4 = dn.rearrange("p (g h s) -> p g h s", g=2, h=NH)
                    dn3 = dn4[:, 0, :, :]
                    rc3 = dn4[:, 1, :, :]
                    nc.vector.tensor_copy(dn3[:, :, :], av4[:, :, :, DH])
                    nc.scalar.activation(
                        rc3[:, :, :], dn3[:, :, :],
                        mybir.ActivationFunctionType.Ln)
                    nc.scalar.activation(
                        rc3[:, :, :], rc3[:, :, :],
                        mybir.ActivationFunctionType.Exp, scale=-1.0)
                    ots = []
                    for s4 in range(4):
                        ot = otp.tile([128, 128], bf16, tag="ot",
                                      name=f"ot{m}_{s4}")
                        for h in range(NH):
                            nc.vector.tensor_scalar(
                                ot[:, h * DH:(h + 1) * DH],
                                av4[:, h, s4, 0:DH],
                                rc3[:, h, s4:s4 + 1], None,
                                op0=mybir.AluOpType.mult)
                        ots.append(ot)
                    combo_ot[m] = ots
                    nc.vector.memset(av[:, :], 0.0)
                return norm

            # ---- schedule ----
            # head: minimal K (first ktile only) + Q(b0,q0), then the rest
            # of K(b0,c0) — lets the first scores/exp start ~2us earlier
            emit_KU(0, 0, 0, 128)
            emit_QU(0, 0)
            emit_KU(0, 0, 128, 512)
            n0 = emit_combo(0, [],
                            [[lambda: emit_KU(0, 1)],
                             [lambda: emit_KU(0, 2)],
                             [lambda: emit_KU(0, 3)], []],
                            vu_list=[(0, k) for k in range(16)])
            n1 = emit_combo(1, [lambda: emit_QU(0, 1)],
                            [[lambda: emit_trans_out(0)], [], [], []],
                            norm_prev=n0)
            n2 = emit_combo(2, [lambda: emit_QU(0, 2)],
                            [[lambda: emit_trans_out(1)],
                             [lambda: emit_KU(1, 0)], [], []],
                            vu_list=[(1, k) for k in range(4)],
                            norm_prev=n1)
            n3 = emit_combo(3, [lambda: emit_QU(0, 3)],
                            [[lambda: emit_trans_out(2)],
                             [lambda: emit_KU(1, 1)],
                             [lambda: emit_KU(1, 2)],
                             [lambda: emit_KU(1, 3)]],
                            vu_list=[(1, k + 4) for k in range(4)],
                            norm_prev=n2)
            n4 = emit_combo(4, [lambda: emit_QU(1, 0)],
                            [[], [], [], []],
                            vu_list=[(1, k + 8) for k in range(8)],
                            norm_prev=n3)
            n5 = emit_combo(5, [lambda: emit_QU(1, 1)],
                            [[lambda: emit_trans_out(3)],
                             [lambda: emit_trans_out(4)], [], []],
                            norm_prev=n4)
            n6 = emit_combo(6, [lambda: emit_QU(1, 2)],
                            [[lambda: emit_trans_out(5)], [], [], []],
                            norm_prev=n5)
            n7 = emit_combo(7, [lambda: emit_QU(1, 3)],
                            [[lambda: emit_trans_out(6)], [], [], []],
                            norm_prev=n6)
            n7()
            emit_trans_out(7)

    nc.compile()
    return nc


def _prep_inputs(x, Wq, bq, Wk, bk, Wv, bv, Wo, bo):
    bf16 = ml_dtypes.bfloat16
    xT = np.ascontiguousarray(
        np.asarray(x, dtype=np.float32).reshape(T, D).T).astype(bf16)
    in_maps = []
    for c in range(NCORES):
        cs = slice(c * 128, (c + 1) * 128)
        in_maps.append({
            "xT": xT,
            "wq": np.ascontiguousarray(Wq[:, cs]).astype(bf16),
            "wk": np.ascontiguousarray(Wk[:, cs]).astype(bf16),
            "wv": np.ascontiguousarray(Wv[:, cs]).astype(bf16),
            "wo": np.ascontiguousarray(Wo[cs, :]).astype(bf16),
            "bq": np.ascontiguousarray(bq[cs]).reshape(128, 1).astype(np.float32),
            "bk": np.ascontiguousarray(bk[cs]).reshape(128, 1).astype(np.float32),
        })
    return in_maps


def kernel(x, Wq, bq, Wk, bk, Wv, bv, Wo, bo, _trace=False, _results=None):
    from concourse.bass_utils import run_bass_kernel_spmd

    x = np.asarray(x); Wq = np.asarray(Wq); Wk = np.asarray(Wk)
    Wv = np.asarray(Wv); Wo = np.asarray(Wo)
    bq = np.asarray(bq); bk = np.asarray(bk); bv = np.asarray(bv)
    bo = np.asarray(bo)

    if "nc" not in _CACHE:
        _CACHE["nc"] = _build_nc()
    nc = _CACHE["nc"]

    in_maps = _prep_inputs(x, Wq, bq, Wk, bk, Wv, bv, Wo, bo)
    res = run_bass_kernel_spmd(
        nc, in_maps, core_ids=list(range(NCORES)), trace=_trace)
    if _results is not None:
        _results.append(res)

    acc = np.zeros((T, D), dtype=np.float32)
    for c in range(NCORES):
        acc += np.asarray(res.results[c]["outp"], dtype=np.float32)
    acc += bv.astype(np.float32) @ Wo.astype(np.float32) + bo.astype(np.float32)
    return acc.reshape(B, S, D)


# revision 26
# speedup vs baseline: 1.0233x; 1.0233x over previous
"""MHA kernel for Trainium2, 8-core tensor-parallel (2 heads per core).

Problem (hardcoded): x [2, 2048, 1024] fp32, Wq/Wk/Wv/Wo [1024, 1024],
bq/bk/bv/bo [1024], H=16 heads, DH=64.  out = MHA(x).

Sharding: heads are split 8 ways (2 heads = 128 proj columns per core).
Each core computes its heads' attention output and a partial output
projection (row-parallel Wo); the host sums the 8 partials and adds the
closed-form bias terms (bv @ Wo + bo).

v2 design (ScalarE-exp is the bottleneck engine at ~147us; everything
else is scheduled to hide under it):
  - scores S^T [128 k, 2h x 512 q] per ktile (two heads concurrently on
    PE row groups 0-63 / 64-127), exp on ScalarE per ktile (N=1024).
  - AV swapped: pt tile [128 k, 128 q] is the STATIONARY operand (full
    128-col array, FWL-eligible), V augmented with a ones column is the
    MOVING operand (N=65) -> out [128 q, 64 dh | denom] token-major with
    the softmax denominator accumulated for free in column 64.
  - normalize = DVE tensor_scalar divide by the per-partition denom.
  - transpose back to dh-major via one [128,128] identity matmul per
    q-subtile (both heads at once), then the usual row-parallel outproj.
  - QKV/V projections are interleaved into the attention phase's PE idle
    slots (PE has ~9us/combo spare under the 18.4us/combo exp pace).
  - partial outputs DMA'd out in bf16; host accumulates in fp32.
"""

import numpy as np
import ml_dtypes

D = 1024
T = 4096          # B*S tokens
S = 2048
B = 2
NH = 2            # heads per core
DH = 64
NCORES = 8
SCALE = 0.125     # 1/sqrt(DH)
NKT = S // 128    # 16 key tiles per batch
NQC = S // 512    # 4 query chunks per batch
NCK = T // 512    # 8 x^T column chunks
VSLOT = DH + 1    # 65: V columns + ones column

_CACHE = {}


def _build_nc(reps=1):
    import concourse.bacc as bacc
    import concourse.mybir as mybir
    import concourse.tile as tile
    from concourse.hw_specs import get_activation_tables as _gat

    # Pin Exp and Ln to the one table set that holds both, so the
    # table-load placement pass emits a single ACT_TABLE_LOAD instead of
    # thrashing between exp_and_others and natural_log every combo.
    def _pinned_tables(arch):
        out = {}
        for k, fns in _gat(arch).items():
            if k != "natural_log_exp_and_others":
                fns = {f for f in fns if f.name not in ("Exp", "Ln")}
            out[k] = fns
        return out
    bacc.get_activation_tables = _pinned_tables

    dt = mybir.dt
    f32, bf16, i32 = dt.float32, dt.bfloat16, dt.int32

    nc = bacc.Bacc("TRN2", target_bir_lowering=False, debug=False,
                   num_devices=NCORES)

    xT = nc.dram_tensor("xT", [D, T], bf16, kind="ExternalInput")
    wq_d = nc.dram_tensor("wq", [D, 128], bf16, kind="ExternalInput")
    wk_d = nc.dram_tensor("wk", [D, 128], bf16, kind="ExternalInput")
    wv_d = nc.dram_tensor("wv", [D, 128], bf16, kind="ExternalInput")
    wo_d = nc.dram_tensor("wo", [128, D], bf16, kind="ExternalInput")
    bq_d = nc.dram_tensor("bq", [128, 1], f32, kind="ExternalInput")
    bk_d = nc.dram_tensor("bk", [128, 1], f32, kind="ExternalInput")
    outp = nc.dram_tensor("outp", [T, D], bf16, kind="ExternalOutput")

    with tile.TileContext(nc) as tc:
      for _rep in range(reps):
        with (
            tc.tile_pool(name="persist", bufs=1) as pp,
            tc.tile_pool(name="pt", bufs=2) as ptp,
            tc.tile_pool(name="ot", bufs=3) as otp,       # onorm_tok [128,128]
            tc.tile_pool(name="otT", bufs=4) as otTp,     # onormT [128,128]
            tc.tile_pool(name="dn", bufs=2) as dnp,       # denom sbuf [128,8]
            tc.tile_pool(name="outsb", bufs=3) as osp,
            tc.tile_pool(name="st_ps", bufs=2, space="PSUM") as stp,   # 4 banks
            tc.tile_pool(name="av_ps", bufs=1, space="PSUM") as avp,   # 2 banks
            tc.tile_pool(name="mm_ps", bufs=2, space="PSUM") as mmp,   # 2 banks
        ):
            # ---- constants / weights / x^T (one DMA per 512-col chunk,
            # ordered so K(b0,c0) can start as early as possible) ----
            wq = pp.tile([128, D], bf16, tag="wq")
            wk = pp.tile([128, D], bf16, tag="wk")
            wv = pp.tile([128, D], bf16, tag="wv")
            wo = pp.tile([128, D], bf16, tag="wo")
            bq = pp.tile([128, 1], f32, tag="bq")
            bk = pp.tile([128, 1], f32, tag="bk")
            xt_all = pp.tile([128, 8 * T], bf16, tag="xt")
            xt4 = xt_all.rearrange("p (d t) -> p d t", d=8)
            xsrc = xT.ap().rearrange("(d p) c -> p d c", p=128)

            def dma_w(w_sb, w_dr):
                nc.sync.dma_start(
                    out=w_sb.rearrange("p (t c) -> p t c", c=128),
                    in_=w_dr.ap().rearrange("(t p) c -> p t c", p=128))

            def dma_chunk(nck):
                cs = slice(nck * 512, (nck + 1) * 512)
                nc.sync.dma_start(out=xt4[:, :, cs], in_=xsrc[:, :, cs])

            dma_chunk(0)
            dma_w(wk, wk_d)
            nc.sync.dma_start(out=bk[:, :], in_=bk_d.ap()[:, :])
            dma_w(wq, wq_d)
            nc.sync.dma_start(out=bq[:, :], in_=bq_d.ap()[:, :])
            dma_chunk(1)
            dma_w(wv, wv_d)
            nc.sync.dma_start(out=wo[:, :], in_=wo_d.ap()[:, :])
            for nck in range(2, NCK):
                dma_chunk(nck)
            ones1 = pp.tile([128, 1], f32, tag="ones1")
            nc.vector.memset(ones1[:, :], 1.0)

            # identity [128,128] bf16 for PE transposes: (f - p) == 0
            idx = pp.tile([128, 128], i32, tag="idx")
            nc.gpsimd.iota(idx[:, :], pattern=[[1, 128]], base=0,
                           channel_multiplier=-1)
            ident = pp.tile([128, 128], bf16, tag="ident")
            nc.vector.tensor_scalar(ident[:, :], idx[:, :], 0, None,
                                    op0=mybir.AluOpType.is_equal)

            # ---- persistent proj outputs ----
            qt = pp.tile([128, T], bf16, tag="qt")
            kt = pp.tile([128, T], bf16, tag="kt")
            vtm = []
            for b in range(B):
                v_sb = pp.tile([128, NH * NKT * VSLOT], bf16, tag=f"v{b}")
                v4 = v_sb.rearrange("p (h k c) -> p h k c", h=NH, k=NKT)
                nc.vector.memset(v4[:, :, :, DH:DH + 1], 1.0)
                vtm.append(v_sb)

            wq3 = wq.rearrange("p (t c) -> p t c", c=128)
            wk3 = wk.rearrange("p (t c) -> p t c", c=128)
            wv3 = wv.rearrange("p (t c) -> p t c", c=128)

            # single persistent AV accumulator (re-zeroed by each combo's
            # deferred norm chain, so the memset always follows the reads)
            av = avp.tile([128, NH * 4 * VSLOT], f32, tag="av")
            av4 = av.rearrange("p (h s c) -> p h s c", h=NH, s=4)
            nc.vector.memset(av[:, :], 0.0)

            # ---- filler work units (each: 8 chained MMs + 1 DVE evac) ----
            def emit_QU(b, qc):
                cs = slice(b * S + qc * 512, b * S + qc * 512 + 512)
                ps = mmp.tile([128, 512], f32, tag="mm",
                              name=f"qproj{b}_{qc}")
                for d in range(8):
                    nc.tensor.matmul(ps[:, :], wq3[:, d, :], xt4[:, d, cs],
                                     start=(d == 0), stop=(d == 7))
                nc.vector.tensor_scalar_add(qt[:, cs], ps[:, :], bq[:, :])

            def emit_KU(b, c, c0=0, c1=512):
                cs = slice(b * S + c * 512 + c0, b * S + c * 512 + c1)
                ps = mmp.tile([128, 512], f32, tag="mm",
                              name=f"kproj{b}_{c}_{c0}")
                for d in range(8):
                    nc.tensor.matmul(ps[:, 0:c1 - c0], wk3[:, d, :],
                                     xt4[:, d, cs],
                                     start=(d == 0), stop=(d == 7))
                nc.vector.tensor_scalar_add(kt[:, cs], ps[:, 0:c1 - c0],
                                            bk[:, :])

            def emit_VU(b, k):
                tok0 = b * S + k * 128
                v4 = vtm[b].rearrange("p (h k c) -> p h k c", h=NH, k=NKT)
                ps = mmp.tile([128, 128], f32, tag="mm",
                              name=f"vproj{b}_{k}")
                for d in range(8):
                    nc.tensor.matmul(ps[:, :], xt4[:, d, tok0:tok0 + 128],
                                     wv3[:, d, :],
                                     start=(d == 0), stop=(d == 7))
                nc.vector.tensor_copy(
                    v4[:, :, k, 0:DH],
                    ps.rearrange("p (h c) -> p h c", h=NH)[:, :, :])

            # ---- micro-task factories: each task is <=1us of PE work,
            # popped one-per-ktile inside emit_combo so filler never forms
            # a multi-us lump between two score matmuls ----
            def KU_tasks(b, c):
                cs = slice(b * S + c * 512, b * S + c * 512 + 512)
                state = {}

                def ta():
                    ps = mmp.tile([128, 512], f32, tag="mm",
                                  name=f"kproj{b}_{c}a")
                    state["ps"] = ps
                    for d in range(4):
                        nc.tensor.matmul(ps[:, :], wk3[:, d, :],
                                         xt4[:, d, cs],
                                         start=(d == 0), stop=False)

                def tb():
                    ps = state["ps"]
                    for d in range(4, 8):
                        nc.tensor.matmul(ps[:, :], wk3[:, d, :],
                                         xt4[:, d, cs],
                                         start=False, stop=(d == 7))
                    nc.vector.tensor_scalar_add(kt[:, cs], ps[:, :],
                                                bk[:, :])
                return [ta, tb]

            def QU_tasks(b, qc):
                cs = slice(b * S + qc * 512, b * S + qc * 512 + 512)
                state = {}

                def ta():
                    ps = mmp.tile([128, 512], f32, tag="mm",
                                  name=f"qproj{b}_{qc}a")
                    state["ps"] = ps
                    for d in range(4):
                        nc.tensor.matmul(ps[:, :], wq3[:, d, :],
                                         xt4[:, d, cs],
                                         start=(d == 0), stop=False)

                def tb():
                    ps = state["ps"]
                    for d in range(4, 8):
                        nc.tensor.matmul(ps[:, :], wq3[:, d, :],
                                         xt4[:, d, cs],
                                         start=False, stop=(d == 7))
                    nc.vector.tensor_scalar_add(qt[:, cs], ps[:, :],
                                                bq[:, :])
                return [ta, tb]

            def VU_task(b, k):
                return lambda: emit_VU(b, k)

            def TO_tasks(m):
                # transpose + outproj + out DMA for combo m, as 5 tasks
                # pipelined so each outproj pair's oT CAST has a full
                # ktile of slack: [T0], [P0,T1], [P1,T2], [P2,T3], [P3]
                b, qc = divmod(m, NQC)
                q0 = b * S + qc * 512
                state = {}

                def mk_T(s4):
                    def t():
                        ots = combo_ot[m]
                        tp = mmp.tile([128, 128], f32, tag="mm",
                                      name=f"tp{m}_{s4}")
                        nc.tensor.matmul(tp[:, :], ots[s4][:, :],
                                         ident[:, :], start=True, stop=True)
                        oT = otTp.tile([128, 128], bf16, tag="otT",
                                       name=f"otT{m}_{s4}")
                        nc.vector.tensor_copy(oT[:, :], tp[:, :])
                        state[s4] = oT
                    return t

                def mk_P(s4):
                    def t():
                        oT = state[s4]
                        for jc in range(2):
                            op = mmp.tile([128, 512], f32, tag="mm",
                                          name=f"op{m}_{s4}_{jc}")
                            nc.tensor.matmul(
                                op[:, :], oT[:, :],
                                wo[:, jc * 512:(jc + 1) * 512],
                                start=True, stop=True)
                            osb = osp.tile([128, 512], bf16, tag="outsb",
                                           name=f"osb{m}_{s4}_{jc}")
                            nc.vector.tensor_copy(osb[:, :], op[:, :])
                            r0 = q0 + s4 * 128
                            nc.sync.dma_start(
                                out=outp.ap()[r0:r0 + 128,
                                              jc * 512:(jc + 1) * 512],
                                in_=osb[:, :])
                        if s4 == 3:
                            combo_ot.pop(m)
                    return t

                def seq(*fs):
                    def t():
                        for f in fs:
                            f()
                    return t

                Ts = [mk_T(s) for s in range(4)]
                Ps = [mk_P(s) for s in range(4)]
                return [Ts[0], seq(Ps[0], Ts[1]), seq(Ps[1], Ts[2]),
                        seq(Ps[2], Ts[3]), Ps[3]]

            # ---- per-combo state ----
            combo_ot = {}    # m -> list of 4 onorm_tok tiles

            def emit_trans_out(m):
                # transpose (both heads at once) + outproj + out DMA for
                # combo m (emitted one combo later)
                b, qc = divmod(m, NQC)
                q0 = b * S + qc * 512
                ots = combo_ot.pop(m)
                oTs = []
                for s4 in range(4):
                    tp = mmp.tile([128, 128], f32, tag="mm",
                                  name=f"tp{m}_{s4}")
                    nc.tensor.matmul(tp[:, :], ots[s4][:, :], ident[:, :],
                                     start=True, stop=True)
                    oT = otTp.tile([128, 128], bf16, tag="otT",
                                   name=f"otT{m}_{s4}")
                    nc.vector.tensor_copy(oT[:, :], tp[:, :])
                    oTs.append(oT)
                for s4 in range(4):
                    oT = oTs[s4]
                    for jc in range(2):
                        op = mmp.tile([128, 512], f32, tag="mm",
                                      name=f"op{m}_{s4}_{jc}")
                        nc.tensor.matmul(
                            op[:, :], oT[:, :],
                            wo[:, jc * 512:(jc + 1) * 512],
                            start=True, stop=True)
                        osb = osp.tile([128, 512], bf16, tag="outsb",
                                       name=f"osb{m}_{s4}_{jc}")
                        nc.vector.tensor_copy(osb[:, :], op[:, :])
                        r0 = q0 + s4 * 128
                        nc.sync.dma_start(
                            out=outp.ap()[r0:r0 + 128,
                                          jc * 512:(jc + 1) * 512],
                            in_=osb[:, :])

            # ---- combo emission ----
            def emit_combo(m, tasks, vu_list=(), norm_prev=None):
                """tasks: micro-task list, one popped per ktile; vu_list:
                V-proj units (b, kt) to spread one-per-ktile; norm_prev:
                the previous combo's deferred norm closure (emitted after
                ktile 1's exp so its Ln/Exp never stall the scalar
                queue).  AV matmuls for ktile k are interleaved right
                after the scores of ktile k+2 (exp(k) is done by then
                thanks to the st double-buffer pacing), so the PE never
                piles a serial AV block between combos."""
                b, qc = divmod(m, NQC)
                q0 = b * S + qc * 512
                v4 = vtm[b].rearrange("p (h k c) -> p h k c", h=NH, k=NKT)
                tasks = list(tasks)
                pt = ptp.tile([128, NH * NKT * 512], bf16, tag="pt",
                              name=f"pt{m}")
                pt3 = pt.rearrange("p (h k q) -> p h k q", h=NH, k=NKT)

                # AV swapped: pt stationary [128k,128q], V|1 moving (N=65).
                # The 8 (h,s4) accumulation groups interleave within shared
                # PSUM banks, so hardware zero-on-start (2KB region
                # granularity) would wipe neighbors: the tile is memset
                # (by the preamble / previous norm chain) and accumulated
                # with start=False throughout.
                def emit_av(kti):
                    for h in range(NH):
                        for s4 in range(4):
                            nc.tensor.matmul(
                                av4[:, h, s4, :],
                                pt3[:, h, kti, s4 * 128:(s4 + 1) * 128],
                                v4[:, h, kti, :],
                                start=False, stop=(kti == NKT - 1),
                                skip_group_check=True)

                for kti in range(NKT):
                    k0 = b * S + kti * 128
                    st = stp.tile([128, 1024], f32, tag="st",
                                  name=f"st{m}_{kti}")
                    for h in range(NH):
                        hp = h * DH
                        nc.tensor.matmul(
                            st[:, h * 512:(h + 1) * 512],
                            kt[hp:hp + DH, k0:k0 + 128],
                            qt[hp:hp + DH, q0:q0 + 512],
                            start=True, stop=True)
                    nc.scalar.activation(
                        pt3[:, :, kti, :], st[:, :],
                        mybir.ActivationFunctionType.Exp, scale=SCALE)
                    if kti == 1 and norm_prev is not None:
                        norm_prev()
                    if kti < len(vu_list):
                        emit_VU(*vu_list[kti])
                    if kti >= 2:
                        emit_av(kti - 2)
                    if tasks:
                        tasks.pop(0)()
                emit_av(NKT - 2)
                emit_av(NKT - 1)
                for t in tasks:
                    t()

                def norm():
                    # denoms -> SBUF, recip via Ln/Exp on ScalarE, DVE
                    # mult, then re-zero the shared AV accumulator for the
                    # next combo (after all reads).
                    dn = dnp.tile([128, 2 * NH * 4], f32, tag="dn",
                                  name=f"dn{m}")
                    dn4 = dn.rearrange("p (g h s) -> p g h s", g=2, h=NH)
                    dn3 = dn4[:, 0, :, :]
                    rc3 = dn4[:, 1, :, :]
                    nc.vector.tensor_copy(dn3[:, :, :], av4[:, :, :, DH])
                    nc.scalar.activation(
                        rc3[:, :, :], dn3[:, :, :],
                        mybir.ActivationFunctionType.Ln)
                    nc.scalar.activation(
                        rc3[:, :, :], rc3[:, :, :],
                        mybir.ActivationFunctionType.Exp, scale=-1.0)
                    ots = []
                    for s4 in range(4):
                        ot = otp.tile([128, 128], bf16, tag="ot",
                                      name=f"ot{m}_{s4}")
                        for h in range(NH):
                            nc.vector.tensor_scalar(
                                ot[:, h * DH:(h + 1) * DH],
                                av4[:, h, s4, 0:DH],
                                rc3[:, h, s4:s4 + 1], None,
                                op0=mybir.AluOpType.mult)
                        ots.append(ot)
                    combo_ot[m] = ots
                    nc.vector.memset(av[:, :], 0.0)
                return norm

            # ---- schedule ----
            # PE warmup while the first x^T chunk DMA is in flight: ~16
            # dummy matmuls on the identity tile ramp the PE p-state so
            # the first projection chain runs at speed.
            for wi in range(16):
                wps = mmp.tile([128, 128], f32, tag="mm",
                               name=f"warm{wi}")
                nc.tensor.matmul(wps[:, :], ident[:, :], ident[:, :],
                                 start=True, stop=True)
            emit_KU(0, 0)
            emit_QU(0, 0)
            n0 = emit_combo(0,
                            KU_tasks(0, 1) + KU_tasks(0, 2)
                            + KU_tasks(0, 3) + QU_tasks(0, 1),
                            vu_list=[(0, k) for k in range(16)])
            n1 = emit_combo(1, QU_tasks(0, 2) + TO_tasks(0),
                            norm_prev=n0)
            n2 = emit_combo(2,
                            QU_tasks(0, 3) + TO_tasks(1) + KU_tasks(1, 0)
                            + [VU_task(1, 0), VU_task(1, 1)],
                            norm_prev=n1)
            n3 = emit_combo(3,
                            QU_tasks(1, 0) + TO_tasks(2) + KU_tasks(1, 1)
                            + KU_tasks(1, 2)
                            + [VU_task(1, k) for k in range(2, 6)],
                            norm_prev=n2)
            n4 = emit_combo(4,
                            QU_tasks(1, 1) + KU_tasks(1, 3)
                            + [VU_task(1, k) for k in range(6, 16)],
                            norm_prev=n3)
            n5 = emit_combo(5, QU_tasks(1, 2) + TO_tasks(3) + TO_tasks(4),
                            norm_prev=n4)
            n6 = emit_combo(6, QU_tasks(1, 3) + TO_tasks(5),
                            norm_prev=n5)
            n7 = emit_combo(7, [lambda: None, lambda: None] + TO_tasks(6),
                            norm_prev=n6)
            n7()
            for t in TO_tasks(7):
                t()

    nc.compile()
    return nc


def _prep_inputs(x, Wq, bq, Wk, bk, Wv, bv, Wo, bo):
    bf16 = ml_dtypes.bfloat16
    xT = np.ascontiguousarray(
        np.asarray(x, dtype=np.float32).reshape(T, D).T).astype(bf16)
    in_maps = []
    for c in range(NCORES):
        cs = slice(c * 128, (c + 1) * 128)
        in_maps.append({
            "xT": xT,
            "wq": np.ascontiguousarray(Wq[:, cs]).astype(bf16),
            "wk": np.ascontiguousarray(Wk[:, cs]).astype(bf16),
            "wv": np.ascontiguousarray(Wv[:, cs]).astype(bf16),
            "wo": np.ascontiguousarray(Wo[cs, :]).astype(bf16),
            "bq": np.ascontiguousarray(bq[cs]).reshape(128, 1).astype(np.float32),
            "bk": np.ascontiguousarray(bk[cs]).reshape(128, 1).astype(np.float32),
        })
    return in_maps


def kernel(x, Wq, bq, Wk, bk, Wv, bv, Wo, bo, _trace=False, _results=None):
    from concourse.bass_utils import run_bass_kernel_spmd

    x = np.asarray(x); Wq = np.asarray(Wq); Wk = np.asarray(Wk)
    Wv = np.asarray(Wv); Wo = np.asarray(Wo)
    bq = np.asarray(bq); bk = np.asarray(bk); bv = np.asarray(bv)
    bo = np.asarray(bo)

    if "nc" not in _CACHE:
        _CACHE["nc"] = _build_nc()
    nc = _CACHE["nc"]

    in_maps = _prep_inputs(x, Wq, bq, Wk, bk, Wv, bv, Wo, bo)
    res = run_bass_kernel_spmd(
        nc, in_maps, core_ids=list(range(NCORES)), trace=_trace)
    if _results is not None:
        _results.append(res)

    acc = np.zeros((T, D), dtype=np.float32)
    for c in range(NCORES):
        acc += np.asarray(res.results[c]["outp"], dtype=np.float32)
    acc += bv.astype(np.float32) @ Wo.astype(np.float32) + bo.astype(np.float32)
    return acc.reshape(B, S, D)


# revision 27
# speedup vs baseline: 1.0250x; 1.0017x over previous
"""MHA kernel for Trainium2, 8-core tensor-parallel (2 heads per core).

Problem (hardcoded): x [2, 2048, 1024] fp32, Wq/Wk/Wv/Wo [1024, 1024],
bq/bk/bv/bo [1024], H=16 heads, DH=64.  out = MHA(x).

Sharding: heads are split 8 ways (2 heads = 128 proj columns per core).
Each core computes its heads' attention output and a partial output
projection (row-parallel Wo); the host sums the 8 partials and adds the
closed-form bias terms (bv @ Wo + bo).

v2 design (ScalarE-exp is the bottleneck engine at ~147us; everything
else is scheduled to hide under it):
  - scores S^T [128 k, 2h x 512 q] per ktile (two heads concurrently on
    PE row groups 0-63 / 64-127), exp on ScalarE per ktile (N=1024).
  - AV swapped: pt tile [128 k, 128 q] is the STATIONARY operand (full
    128-col array, FWL-eligible), V augmented with a ones column is the
    MOVING operand (N=65) -> out [128 q, 64 dh | denom] token-major with
    the softmax denominator accumulated for free in column 64.
  - normalize = DVE tensor_scalar divide by the per-partition denom.
  - transpose back to dh-major via one [128,128] identity matmul per
    q-subtile (both heads at once), then the usual row-parallel outproj.
  - QKV/V projections are interleaved into the attention phase's PE idle
    slots (PE has ~9us/combo spare under the 18.4us/combo exp pace).
  - partial outputs DMA'd out in bf16; host accumulates in fp32.
"""

import numpy as np
import ml_dtypes

D = 1024
T = 4096          # B*S tokens
S = 2048
B = 2
NH = 2            # heads per core
DH = 64
NCORES = 8
SCALE = 0.125     # 1/sqrt(DH)
NKT = S // 128    # 16 key tiles per batch
NQC = S // 512    # 4 query chunks per batch
NCK = T // 512    # 8 x^T column chunks
VSLOT = DH + 1    # 65: V columns + ones column

_CACHE = {}


def _build_nc(reps=1):
    import concourse.bacc as bacc
    import concourse.mybir as mybir
    import concourse.tile as tile
    from concourse.hw_specs import get_activation_tables as _gat

    # Pin Exp and Ln to the one table set that holds both, so the
    # table-load placement pass emits a single ACT_TABLE_LOAD instead of
    # thrashing between exp_and_others and natural_log every combo.
    def _pinned_tables(arch):
        out = {}
        for k, fns in _gat(arch).items():
            if k != "natural_log_exp_and_others":
                fns = {f for f in fns if f.name not in ("Exp", "Ln")}
            out[k] = fns
        return out
    bacc.get_activation_tables = _pinned_tables

    dt = mybir.dt
    f32, bf16, i32 = dt.float32, dt.bfloat16, dt.int32

    nc = bacc.Bacc("TRN2", target_bir_lowering=False, debug=False,
                   num_devices=NCORES)

    xT = nc.dram_tensor("xT", [D, T], bf16, kind="ExternalInput")
    wq_d = nc.dram_tensor("wq", [D, 128], bf16, kind="ExternalInput")
    wk_d = nc.dram_tensor("wk", [D, 128], bf16, kind="ExternalInput")
    wv_d = nc.dram_tensor("wv", [D, 128], bf16, kind="ExternalInput")
    wo_d = nc.dram_tensor("wo", [128, D], bf16, kind="ExternalInput")
    bq_d = nc.dram_tensor("bq", [128, 1], f32, kind="ExternalInput")
    bk_d = nc.dram_tensor("bk", [128, 1], f32, kind="ExternalInput")
    outp = nc.dram_tensor("outp", [T, D], bf16, kind="ExternalOutput")

    with tile.TileContext(nc) as tc:
      for _rep in range(reps):
        with (
            tc.tile_pool(name="persist", bufs=1) as pp,
            tc.tile_pool(name="pt", bufs=2) as ptp,
            tc.tile_pool(name="ot", bufs=3) as otp,       # onorm_tok [128,128]
            tc.tile_pool(name="otT", bufs=4) as otTp,     # onormT [128,128]
            tc.tile_pool(name="dn", bufs=2) as dnp,       # denom sbuf [128,8]
            tc.tile_pool(name="outsb", bufs=3) as osp,
            tc.tile_pool(name="st_ps", bufs=2, space="PSUM") as stp,   # 4 banks
            tc.tile_pool(name="av_ps", bufs=1, space="PSUM") as avp,   # 2 banks
            tc.tile_pool(name="mm_ps", bufs=2, space="PSUM") as mmp,   # 2 banks
        ):
            # ---- constants / weights / x^T (one DMA per 512-col chunk,
            # ordered so K(b0,c0) can start as early as possible) ----
            wq = pp.tile([128, D], bf16, tag="wq")
            wk = pp.tile([128, D], bf16, tag="wk")
            wv = pp.tile([128, D], bf16, tag="wv")
            wo = pp.tile([128, D], bf16, tag="wo")
            bq = pp.tile([128, 1], f32, tag="bq")
            bk = pp.tile([128, 1], f32, tag="bk")
            xt_all = pp.tile([128, 8 * T], bf16, tag="xt")
            xt4 = xt_all.rearrange("p (d t) -> p d t", d=8)
            xsrc = xT.ap().rearrange("(d p) c -> p d c", p=128)

            def dma_w(w_sb, w_dr):
                nc.sync.dma_start(
                    out=w_sb.rearrange("p (t c) -> p t c", c=128),
                    in_=w_dr.ap().rearrange("(t p) c -> p t c", p=128))

            def dma_chunk(nck):
                cs = slice(nck * 512, (nck + 1) * 512)
                nc.sync.dma_start(out=xt4[:, :, cs], in_=xsrc[:, :, cs])

            dma_chunk(0)
            dma_w(wk, wk_d)
            nc.sync.dma_start(out=bk[:, :], in_=bk_d.ap()[:, :])
            dma_w(wq, wq_d)
            nc.sync.dma_start(out=bq[:, :], in_=bq_d.ap()[:, :])
            dma_chunk(1)
            dma_w(wv, wv_d)
            nc.sync.dma_start(out=wo[:, :], in_=wo_d.ap()[:, :])
            for nck in range(2, NCK):
                dma_chunk(nck)
            ones1 = pp.tile([128, 1], f32, tag="ones1")
            nc.vector.memset(ones1[:, :], 1.0)

            # identity [128,128] bf16 for PE transposes: (f - p) == 0
            idx = pp.tile([128, 128], i32, tag="idx")
            nc.gpsimd.iota(idx[:, :], pattern=[[1, 128]], base=0,
                           channel_multiplier=-1)
            ident = pp.tile([128, 128], bf16, tag="ident")
            nc.vector.tensor_scalar(ident[:, :], idx[:, :], 0, None,
                                    op0=mybir.AluOpType.is_equal)

            # ---- persistent proj outputs ----
            qt = pp.tile([128, T], bf16, tag="qt")
            kt = pp.tile([128, T], bf16, tag="kt")
            vtm = []
            for b in range(B):
                v_sb = pp.tile([128, NH * NKT * VSLOT], bf16, tag=f"v{b}")
                v4 = v_sb.rearrange("p (h k c) -> p h k c", h=NH, k=NKT)
                nc.vector.memset(v4[:, :, :, DH:DH + 1], 1.0)
                vtm.append(v_sb)

            wq3 = wq.rearrange("p (t c) -> p t c", c=128)
            wk3 = wk.rearrange("p (t c) -> p t c", c=128)
            wv3 = wv.rearrange("p (t c) -> p t c", c=128)

            # single persistent AV accumulator (re-zeroed by each combo's
            # deferred norm chain, so the memset always follows the reads)
            av = avp.tile([128, NH * 4 * VSLOT], f32, tag="av")
            av4 = av.rearrange("p (h s c) -> p h s c", h=NH, s=4)
            nc.vector.memset(av[:, :], 0.0)

            # ---- filler work units (each: 8 chained MMs + 1 DVE evac) ----
            def emit_QU(b, qc):
                cs = slice(b * S + qc * 512, b * S + qc * 512 + 512)
                ps = mmp.tile([128, 512], f32, tag="mm",
                              name=f"qproj{b}_{qc}")
                for d in range(8):
                    nc.tensor.matmul(ps[:, :], wq3[:, d, :], xt4[:, d, cs],
                                     start=(d == 0), stop=(d == 7))
                nc.vector.tensor_scalar_add(qt[:, cs], ps[:, :], bq[:, :])

            def emit_KU(b, c, c0=0, c1=512):
                cs = slice(b * S + c * 512 + c0, b * S + c * 512 + c1)
                ps = mmp.tile([128, 512], f32, tag="mm",
                              name=f"kproj{b}_{c}_{c0}")
                for d in range(8):
                    nc.tensor.matmul(ps[:, 0:c1 - c0], wk3[:, d, :],
                                     xt4[:, d, cs],
                                     start=(d == 0), stop=(d == 7))
                nc.vector.tensor_scalar_add(kt[:, cs], ps[:, 0:c1 - c0],
                                            bk[:, :])

            def emit_VU(b, k):
                tok0 = b * S + k * 128
                v4 = vtm[b].rearrange("p (h k c) -> p h k c", h=NH, k=NKT)
                ps = mmp.tile([128, 128], f32, tag="mm",
                              name=f"vproj{b}_{k}")
                for d in range(8):
                    nc.tensor.matmul(ps[:, :], xt4[:, d, tok0:tok0 + 128],
                                     wv3[:, d, :],
                                     start=(d == 0), stop=(d == 7))
                nc.vector.tensor_copy(
                    v4[:, :, k, 0:DH],
                    ps.rearrange("p (h c) -> p h c", h=NH)[:, :, :])

            # ---- micro-task factories: each task is <=1us of PE work,
            # popped one-per-ktile inside emit_combo so filler never forms
            # a multi-us lump between two score matmuls ----
            def KU_tasks(b, c):
                cs = slice(b * S + c * 512, b * S + c * 512 + 512)
                state = {}

                def ta():
                    ps = mmp.tile([128, 512], f32, tag="mm",
                                  name=f"kproj{b}_{c}a")
                    state["ps"] = ps
                    for d in range(4):
                        nc.tensor.matmul(ps[:, :], wk3[:, d, :],
                                         xt4[:, d, cs],
                                         start=(d == 0), stop=False)

                def tb():
                    ps = state["ps"]
                    for d in range(4, 8):
                        nc.tensor.matmul(ps[:, :], wk3[:, d, :],
                                         xt4[:, d, cs],
                                         start=False, stop=(d == 7))
                    nc.vector.tensor_scalar_add(kt[:, cs], ps[:, :],
                                                bk[:, :])
                return [ta, tb]

            def QU_tasks(b, qc):
                cs = slice(b * S + qc * 512, b * S + qc * 512 + 512)
                state = {}

                def ta():
                    ps = mmp.tile([128, 512], f32, tag="mm",
                                  name=f"qproj{b}_{qc}a")
                    state["ps"] = ps
                    for d in range(4):
                        nc.tensor.matmul(ps[:, :], wq3[:, d, :],
                                         xt4[:, d, cs],
                                         start=(d == 0), stop=False)

                def tb():
                    ps = state["ps"]
                    for d in range(4, 8):
                        nc.tensor.matmul(ps[:, :], wq3[:, d, :],
                                         xt4[:, d, cs],
                                         start=False, stop=(d == 7))
                    nc.vector.tensor_scalar_add(qt[:, cs], ps[:, :],
                                                bq[:, :])
                return [ta, tb]

            def VU_task(b, k):
                return lambda: emit_VU(b, k)

            def TO_tasks(m, tail=False):
                # transpose + outproj + out DMA for combo m, as 5 tasks
                # pipelined so each outproj pair's oT CAST has a full
                # ktile of slack: [T0], [P0,T1], [P1,T2], [P2,T3], [P3].
                # tail=True routes half the output casts to the (by then
                # idle) ScalarE so the final drain is not DVE-serial.
                b, qc = divmod(m, NQC)
                q0 = b * S + qc * 512
                state = {}

                def mk_T(s4):
                    def t():
                        ots = combo_ot[m]
                        tp = mmp.tile([128, 128], f32, tag="mm",
                                      name=f"tp{m}_{s4}")
                        nc.tensor.matmul(tp[:, :], ots[s4][:, :],
                                         ident[:, :], start=True, stop=True)
                        oT = otTp.tile([128, 128], bf16, tag="otT",
                                       name=f"otT{m}_{s4}")
                        nc.vector.tensor_copy(oT[:, :], tp[:, :])
                        state[s4] = oT
                    return t

                def mk_P(s4):
                    def t():
                        oT = state[s4]
                        for jc in range(2):
                            op = mmp.tile([128, 512], f32, tag="mm",
                                          name=f"op{m}_{s4}_{jc}")
                            nc.tensor.matmul(
                                op[:, :], oT[:, :],
                                wo[:, jc * 512:(jc + 1) * 512],
                                start=True, stop=True)
                            osb = osp.tile([128, 512], bf16, tag="outsb",
                                           name=f"osb{m}_{s4}_{jc}")
                            if tail and jc == 1:
                                nc.scalar.copy(osb[:, :], op[:, :])
                            else:
                                nc.vector.tensor_copy(osb[:, :], op[:, :])
                            r0 = q0 + s4 * 128
                            nc.sync.dma_start(
                                out=outp.ap()[r0:r0 + 128,
                                              jc * 512:(jc + 1) * 512],
                                in_=osb[:, :])
                        if s4 == 3:
                            combo_ot.pop(m)
                    return t

                def seq(*fs):
                    def t():
                        for f in fs:
                            f()
                    return t

                Ts = [mk_T(s) for s in range(4)]
                Ps = [mk_P(s) for s in range(4)]
                return [Ts[0], seq(Ps[0], Ts[1]), seq(Ps[1], Ts[2]),
                        seq(Ps[2], Ts[3]), Ps[3]]

            # ---- per-combo state ----
            combo_ot = {}    # m -> list of 4 onorm_tok tiles

            def emit_trans_out(m):
                # transpose (both heads at once) + outproj + out DMA for
                # combo m (emitted one combo later)
                b, qc = divmod(m, NQC)
                q0 = b * S + qc * 512
                ots = combo_ot.pop(m)
                oTs = []
                for s4 in range(4):
                    tp = mmp.tile([128, 128], f32, tag="mm",
                                  name=f"tp{m}_{s4}")
                    nc.tensor.matmul(tp[:, :], ots[s4][:, :], ident[:, :],
                                     start=True, stop=True)
                    oT = otTp.tile([128, 128], bf16, tag="otT",
                                   name=f"otT{m}_{s4}")
                    nc.vector.tensor_copy(oT[:, :], tp[:, :])
                    oTs.append(oT)
                for s4 in range(4):
                    oT = oTs[s4]
                    for jc in range(2):
                        op = mmp.tile([128, 512], f32, tag="mm",
                                      name=f"op{m}_{s4}_{jc}")
                        nc.tensor.matmul(
                            op[:, :], oT[:, :],
                            wo[:, jc * 512:(jc + 1) * 512],
                            start=True, stop=True)
                        osb = osp.tile([128, 512], bf16, tag="outsb",
                                       name=f"osb{m}_{s4}_{jc}")
                        nc.vector.tensor_copy(osb[:, :], op[:, :])
                        r0 = q0 + s4 * 128
                        nc.sync.dma_start(
                            out=outp.ap()[r0:r0 + 128,
                                          jc * 512:(jc + 1) * 512],
                            in_=osb[:, :])

            # ---- combo emission ----
            def emit_combo(m, tasks, vu_list=(), norm_prev=None):
                """tasks: micro-task list, one popped per ktile; vu_list:
                V-proj units (b, kt) to spread one-per-ktile; norm_prev:
                the previous combo's deferred norm closure (emitted after
                ktile 1's exp so its Ln/Exp never stall the scalar
                queue).  AV matmuls for ktile k are interleaved right
                after the scores of ktile k+2 (exp(k) is done by then
                thanks to the st double-buffer pacing), so the PE never
                piles a serial AV block between combos."""
                b, qc = divmod(m, NQC)
                q0 = b * S + qc * 512
                v4 = vtm[b].rearrange("p (h k c) -> p h k c", h=NH, k=NKT)
                tasks = list(tasks)
                pt = ptp.tile([128, NH * NKT * 512], bf16, tag="pt",
                              name=f"pt{m}")
                pt3 = pt.rearrange("p (h k q) -> p h k q", h=NH, k=NKT)

                # AV swapped: pt stationary [128k,128q], V|1 moving (N=65).
                # The 8 (h,s4) accumulation groups interleave within shared
                # PSUM banks, so hardware zero-on-start (2KB region
                # granularity) would wipe neighbors: the tile is memset
                # (by the preamble / previous norm chain) and accumulated
                # with start=False throughout.
                def emit_av(kti):
                    for h in range(NH):
                        for s4 in range(4):
                            nc.tensor.matmul(
                                av4[:, h, s4, :],
                                pt3[:, h, kti, s4 * 128:(s4 + 1) * 128],
                                v4[:, h, kti, :],
                                start=False, stop=(kti == NKT - 1),
                                skip_group_check=True)

                for kti in range(NKT):
                    k0 = b * S + kti * 128
                    st = stp.tile([128, 1024], f32, tag="st",
                                  name=f"st{m}_{kti}")
                    for h in range(NH):
                        hp = h * DH
                        nc.tensor.matmul(
                            st[:, h * 512:(h + 1) * 512],
                            kt[hp:hp + DH, k0:k0 + 128],
                            qt[hp:hp + DH, q0:q0 + 512],
                            start=True, stop=True)
                    nc.scalar.activation(
                        pt3[:, :, kti, :], st[:, :],
                        mybir.ActivationFunctionType.Exp, scale=SCALE)
                    if kti == 1 and norm_prev is not None:
                        norm_prev()
                    if kti < len(vu_list):
                        emit_VU(*vu_list[kti])
                    if kti >= 2:
                        emit_av(kti - 2)
                    if tasks:
                        tasks.pop(0)()
                emit_av(NKT - 2)
                emit_av(NKT - 1)
                for t in tasks:
                    t()

                def norm():
                    # denoms -> SBUF, recip via Ln/Exp on ScalarE, DVE
                    # mult, then re-zero the shared AV accumulator for the
                    # next combo (after all reads).
                    dn = dnp.tile([128, 2 * NH * 4], f32, tag="dn",
                                  name=f"dn{m}")
                    dn4 = dn.rearrange("p (g h s) -> p g h s", g=2, h=NH)
                    dn3 = dn4[:, 0, :, :]
                    rc3 = dn4[:, 1, :, :]
                    nc.vector.tensor_copy(dn3[:, :, :], av4[:, :, :, DH])
                    nc.scalar.activation(
                        rc3[:, :, :], dn3[:, :, :],
                        mybir.ActivationFunctionType.Ln)
                    nc.scalar.activation(
                        rc3[:, :, :], rc3[:, :, :],
                        mybir.ActivationFunctionType.Exp, scale=-1.0)
                    ots = []
                    for s4 in range(4):
                        ot = otp.tile([128, 128], bf16, tag="ot",
                                      name=f"ot{m}_{s4}")
                        for h in range(NH):
                            nc.vector.tensor_scalar(
                                ot[:, h * DH:(h + 1) * DH],
                                av4[:, h, s4, 0:DH],
                                rc3[:, h, s4:s4 + 1], None,
                                op0=mybir.AluOpType.mult)
                        ots.append(ot)
                    combo_ot[m] = ots
                    nc.vector.memset(av[:, :], 0.0)
                return norm

            # ---- schedule ----
            # PE warmup while the first x^T chunk DMA is in flight: ~16
            # dummy matmuls on the identity tile ramp the PE p-state so
            # the first projection chain runs at speed.
            for wi in range(24):
                wps = mmp.tile([128, 128], f32, tag="mm",
                               name=f"warm{wi}")
                nc.tensor.matmul(wps[:, :], ident[:, :], ident[:, :],
                                 start=True, stop=True)
            emit_KU(0, 0)
            emit_QU(0, 0)
            n0 = emit_combo(0,
                            KU_tasks(0, 1) + KU_tasks(0, 2)
                            + KU_tasks(0, 3) + QU_tasks(0, 1),
                            vu_list=[(0, k) for k in range(16)])
            n1 = emit_combo(1, QU_tasks(0, 2) + TO_tasks(0),
                            norm_prev=n0)
            n2 = emit_combo(2,
                            QU_tasks(0, 3) + TO_tasks(1) + KU_tasks(1, 0)
                            + [VU_task(1, 0), VU_task(1, 1)],
                            norm_prev=n1)
            n3 = emit_combo(3,
                            QU_tasks(1, 0) + TO_tasks(2) + KU_tasks(1, 1)
                            + KU_tasks(1, 2)
                            + [VU_task(1, k) for k in range(2, 6)],
                            norm_prev=n2)
            n4 = emit_combo(4,
                            QU_tasks(1, 1) + KU_tasks(1, 3)
                            + [VU_task(1, k) for k in range(6, 16)],
                            norm_prev=n3)
            n5 = emit_combo(5, QU_tasks(1, 2) + TO_tasks(3) + TO_tasks(4),
                            norm_prev=n4)
            n6 = emit_combo(6, QU_tasks(1, 3) + TO_tasks(5),
                            norm_prev=n5)
            n7 = emit_combo(7, [lambda: None, lambda: None] + TO_tasks(6),
                            norm_prev=n6)
            n7()
            for t in TO_tasks(7, tail=True):
                t()

    nc.compile()
    return nc


def _prep_inputs(x, Wq, bq, Wk, bk, Wv, bv, Wo, bo):
    bf16 = ml_dtypes.bfloat16
    xT = np.ascontiguousarray(
        np.asarray(x, dtype=np.float32).reshape(T, D).T).astype(bf16)
    in_maps = []
    for c in range(NCORES):
        cs = slice(c * 128, (c + 1) * 128)
        in_maps.append({
            "xT": xT,
            "wq": np.ascontiguousarray(Wq[:, cs]).astype(bf16),
            "wk": np.ascontiguousarray(Wk[:, cs]).astype(bf16),
            "wv": np.ascontiguousarray(Wv[:, cs]).astype(bf16),
            "wo": np.ascontiguousarray(Wo[cs, :]).astype(bf16),
            "bq": np.ascontiguousarray(bq[cs]).reshape(128, 1).astype(np.float32),
            "bk": np.ascontiguousarray(bk[cs]).reshape(128, 1).astype(np.float32),
        })
    return in_maps


def kernel(x, Wq, bq, Wk, bk, Wv, bv, Wo, bo, _trace=False, _results=None):
    from concourse.bass_utils import run_bass_kernel_spmd

    x = np.asarray(x); Wq = np.asarray(Wq); Wk = np.asarray(Wk)
    Wv = np.asarray(Wv); Wo = np.asarray(Wo)
    bq = np.asarray(bq); bk = np.asarray(bk); bv = np.asarray(bv)
    bo = np.asarray(bo)

    if "nc" not in _CACHE:
        _CACHE["nc"] = _build_nc()
    nc = _CACHE["nc"]

    in_maps = _prep_inputs(x, Wq, bq, Wk, bk, Wv, bv, Wo, bo)
    res = run_bass_kernel_spmd(
        nc, in_maps, core_ids=list(range(NCORES)), trace=_trace)
    if _results is not None:
        _results.append(res)

    acc = np.zeros((T, D), dtype=np.float32)
    for c in range(NCORES):
        acc += np.asarray(res.results[c]["outp"], dtype=np.float32)
    acc += bv.astype(np.float32) @ Wo.astype(np.float32) + bo.astype(np.float32)
    return acc.reshape(B, S, D)


# revision 28
# speedup vs baseline: 1.0258x; 1.0008x over previous
"""MHA kernel for Trainium2, 8-core tensor-parallel (2 heads per core).

Problem (hardcoded): x [2, 2048, 1024] fp32, Wq/Wk/Wv/Wo [1024, 1024],
bq/bk/bv/bo [1024], H=16 heads, DH=64.  out = MHA(x).

Sharding: heads are split 8 ways (2 heads = 128 proj columns per core).
Each core computes its heads' attention output and a partial output
projection (row-parallel Wo); the host sums the 8 partials and adds the
closed-form bias terms (bv @ Wo + bo).

v2 design (ScalarE-exp is the bottleneck engine at ~147us; everything
else is scheduled to hide under it):
  - scores S^T [128 k, 2h x 512 q] per ktile (two heads concurrently on
    PE row groups 0-63 / 64-127), exp on ScalarE per ktile (N=1024).
  - AV swapped: pt tile [128 k, 128 q] is the STATIONARY operand (full
    128-col array, FWL-eligible), V augmented with a ones column is the
    MOVING operand (N=65) -> out [128 q, 64 dh | denom] token-major with
    the softmax denominator accumulated for free in column 64.
  - normalize = DVE tensor_scalar divide by the per-partition denom.
  - transpose back to dh-major via one [128,128] identity matmul per
    q-subtile (both heads at once), then the usual row-parallel outproj.
  - QKV/V projections are interleaved into the attention phase's PE idle
    slots (PE has ~9us/combo spare under the 18.4us/combo exp pace).
  - partial outputs DMA'd out in bf16; host accumulates in fp32.
"""

import numpy as np
import ml_dtypes

D = 1024
T = 4096          # B*S tokens
S = 2048
B = 2
NH = 2            # heads per core
DH = 64
NCORES = 8
SCALE = 0.125     # 1/sqrt(DH)
NKT = S // 128    # 16 key tiles per batch
NQC = S // 512    # 4 query chunks per batch
NCK = T // 512    # 8 x^T column chunks
VSLOT = DH + 1    # 65: V columns + ones column

_CACHE = {}


def _build_nc(reps=1):
    import concourse.bacc as bacc
    import concourse.mybir as mybir
    import concourse.tile as tile
    from concourse.hw_specs import get_activation_tables as _gat

    # Pin Exp and Ln to the one table set that holds both, so the
    # table-load placement pass emits a single ACT_TABLE_LOAD instead of
    # thrashing between exp_and_others and natural_log every combo.
    def _pinned_tables(arch):
        out = {}
        for k, fns in _gat(arch).items():
            if k != "natural_log_exp_and_others":
                fns = {f for f in fns if f.name not in ("Exp", "Ln")}
            out[k] = fns
        return out
    bacc.get_activation_tables = _pinned_tables

    dt = mybir.dt
    f32, bf16, i32 = dt.float32, dt.bfloat16, dt.int32

    nc = bacc.Bacc("TRN2", target_bir_lowering=False, debug=False,
                   num_devices=NCORES)

    xT = nc.dram_tensor("xT", [D, T], bf16, kind="ExternalInput")
    wq_d = nc.dram_tensor("wq", [D, 128], bf16, kind="ExternalInput")
    wk_d = nc.dram_tensor("wk", [D, 128], bf16, kind="ExternalInput")
    wv_d = nc.dram_tensor("wv", [D, 128], bf16, kind="ExternalInput")
    wo_d = nc.dram_tensor("wo", [128, D], bf16, kind="ExternalInput")
    bq_d = nc.dram_tensor("bq", [128, 1], f32, kind="ExternalInput")
    bk_d = nc.dram_tensor("bk", [128, 1], f32, kind="ExternalInput")
    outp = nc.dram_tensor("outp", [T, D], bf16, kind="ExternalOutput")

    with tile.TileContext(nc) as tc:
      for _rep in range(reps):
        with (
            tc.tile_pool(name="persist", bufs=1) as pp,
            tc.tile_pool(name="pt", bufs=2) as ptp,
            tc.tile_pool(name="ot", bufs=3) as otp,       # onorm_tok [128,128]
            tc.tile_pool(name="otT", bufs=4) as otTp,     # onormT [128,128]
            tc.tile_pool(name="dn", bufs=2) as dnp,       # denom sbuf [128,8]
            tc.tile_pool(name="outsb", bufs=3) as osp,
            tc.tile_pool(name="st_ps", bufs=2, space="PSUM") as stp,   # 4 banks
            tc.tile_pool(name="av_ps", bufs=1, space="PSUM") as avp,   # 2 banks
            tc.tile_pool(name="mm_ps", bufs=2, space="PSUM") as mmp,   # 2 banks
        ):
            # ---- constants / weights / x^T (one DMA per 512-col chunk,
            # ordered so K(b0,c0) can start as early as possible) ----
            wq = pp.tile([128, D], bf16, tag="wq")
            wk = pp.tile([128, D], bf16, tag="wk")
            wv = pp.tile([128, D], bf16, tag="wv")
            wo = pp.tile([128, D], bf16, tag="wo")
            bq = pp.tile([128, 1], f32, tag="bq")
            bk = pp.tile([128, 1], f32, tag="bk")
            xt_all = pp.tile([128, 8 * T], bf16, tag="xt")
            xt4 = xt_all.rearrange("p (d t) -> p d t", d=8)
            xsrc = xT.ap().rearrange("(d p) c -> p d c", p=128)

            def dma_w(w_sb, w_dr):
                nc.sync.dma_start(
                    out=w_sb.rearrange("p (t c) -> p t c", c=128),
                    in_=w_dr.ap().rearrange("(t p) c -> p t c", p=128))

            def dma_chunk(nck):
                cs = slice(nck * 512, (nck + 1) * 512)
                nc.sync.dma_start(out=xt4[:, :, cs], in_=xsrc[:, :, cs])

            dma_chunk(0)
            dma_w(wk, wk_d)
            nc.sync.dma_start(out=bk[:, :], in_=bk_d.ap()[:, :])
            dma_w(wq, wq_d)
            nc.sync.dma_start(out=bq[:, :], in_=bq_d.ap()[:, :])
            dma_chunk(1)
            dma_w(wv, wv_d)
            nc.sync.dma_start(out=wo[:, :], in_=wo_d.ap()[:, :])
            for nck in range(2, NCK):
                dma_chunk(nck)
            ones1 = pp.tile([128, 1], f32, tag="ones1")
            nc.vector.memset(ones1[:, :], 1.0)

            # identity [128,128] bf16 for PE transposes: (f - p) == 0
            idx = pp.tile([128, 128], i32, tag="idx")
            nc.gpsimd.iota(idx[:, :], pattern=[[1, 128]], base=0,
                           channel_multiplier=-1)
            ident = pp.tile([128, 128], bf16, tag="ident")
            nc.vector.tensor_scalar(ident[:, :], idx[:, :], 0, None,
                                    op0=mybir.AluOpType.is_equal)

            # ---- persistent proj outputs ----
            qt = pp.tile([128, T], bf16, tag="qt")
            kt = pp.tile([128, T], bf16, tag="kt")
            vtm = []
            for b in range(B):
                v_sb = pp.tile([128, NH * NKT * VSLOT], bf16, tag=f"v{b}")
                v4 = v_sb.rearrange("p (h k c) -> p h k c", h=NH, k=NKT)
                nc.vector.memset(v4[:, :, :, DH:DH + 1], 1.0)
                vtm.append(v_sb)

            wq3 = wq.rearrange("p (t c) -> p t c", c=128)
            wk3 = wk.rearrange("p (t c) -> p t c", c=128)
            wv3 = wv.rearrange("p (t c) -> p t c", c=128)

            # single persistent AV accumulator (re-zeroed by each combo's
            # deferred norm chain, so the memset always follows the reads)
            av = avp.tile([128, NH * 4 * VSLOT], f32, tag="av")
            av4 = av.rearrange("p (h s c) -> p h s c", h=NH, s=4)
            nc.vector.memset(av[:, :], 0.0)

            # ---- filler work units (each: 8 chained MMs + 1 DVE evac) ----
            def emit_QU(b, qc):
                cs = slice(b * S + qc * 512, b * S + qc * 512 + 512)
                ps = mmp.tile([128, 512], f32, tag="mm",
                              name=f"qproj{b}_{qc}")
                for d in range(8):
                    nc.tensor.matmul(ps[:, :], wq3[:, d, :], xt4[:, d, cs],
                                     start=(d == 0), stop=(d == 7))
                nc.vector.tensor_scalar_add(qt[:, cs], ps[:, :], bq[:, :])

            def emit_KU(b, c, c0=0, c1=512):
                cs = slice(b * S + c * 512 + c0, b * S + c * 512 + c1)
                ps = mmp.tile([128, 512], f32, tag="mm",
                              name=f"kproj{b}_{c}_{c0}")
                for d in range(8):
                    nc.tensor.matmul(ps[:, 0:c1 - c0], wk3[:, d, :],
                                     xt4[:, d, cs],
                                     start=(d == 0), stop=(d == 7))
                nc.vector.tensor_scalar_add(kt[:, cs], ps[:, 0:c1 - c0],
                                            bk[:, :])

            def emit_VU(b, k):
                tok0 = b * S + k * 128
                v4 = vtm[b].rearrange("p (h k c) -> p h k c", h=NH, k=NKT)
                ps = mmp.tile([128, 128], f32, tag="mm",
                              name=f"vproj{b}_{k}")
                for d in range(8):
                    nc.tensor.matmul(ps[:, :], xt4[:, d, tok0:tok0 + 128],
                                     wv3[:, d, :],
                                     start=(d == 0), stop=(d == 7))
                nc.vector.tensor_copy(
                    v4[:, :, k, 0:DH],
                    ps.rearrange("p (h c) -> p h c", h=NH)[:, :, :])

            # ---- micro-task factories: each task is <=1us of PE work,
            # popped one-per-ktile inside emit_combo so filler never forms
            # a multi-us lump between two score matmuls ----
            def KU_tasks(b, c):
                cs = slice(b * S + c * 512, b * S + c * 512 + 512)
                state = {}

                def ta():
                    ps = mmp.tile([128, 512], f32, tag="mm",
                                  name=f"kproj{b}_{c}a")
                    state["ps"] = ps
                    for d in range(4):
                        nc.tensor.matmul(ps[:, :], wk3[:, d, :],
                                         xt4[:, d, cs],
                                         start=(d == 0), stop=False)

                def tb():
                    ps = state["ps"]
                    for d in range(4, 8):
                        nc.tensor.matmul(ps[:, :], wk3[:, d, :],
                                         xt4[:, d, cs],
                                         start=False, stop=(d == 7))
                    nc.vector.tensor_scalar_add(kt[:, cs], ps[:, :],
                                                bk[:, :])
                return [ta, tb]

            def QU_tasks(b, qc):
                cs = slice(b * S + qc * 512, b * S + qc * 512 + 512)
                state = {}

                def ta():
                    ps = mmp.tile([128, 512], f32, tag="mm",
                                  name=f"qproj{b}_{qc}a")
                    state["ps"] = ps
                    for d in range(4):
                        nc.tensor.matmul(ps[:, :], wq3[:, d, :],
                                         xt4[:, d, cs],
                                         start=(d == 0), stop=False)

                def tb():
                    ps = state["ps"]
                    for d in range(4, 8):
                        nc.tensor.matmul(ps[:, :], wq3[:, d, :],
                                         xt4[:, d, cs],
                                         start=False, stop=(d == 7))
                    nc.vector.tensor_scalar_add(qt[:, cs], ps[:, :],
                                                bq[:, :])
                return [ta, tb]

            def VU_task(b, k):
                return lambda: emit_VU(b, k)

            def TO_tasks(m, tail=False):
                # transpose + outproj + out DMA for combo m, as 5 tasks
                # pipelined so each outproj pair's oT CAST has a full
                # ktile of slack: [T0], [P0,T1], [P1,T2], [P2,T3], [P3].
                # tail=True routes half the output casts to the (by then
                # idle) ScalarE so the final drain is not DVE-serial.
                b, qc = divmod(m, NQC)
                q0 = b * S + qc * 512
                state = {}

                def mk_T(s4):
                    def t():
                        ots = combo_ot[m]
                        tp = mmp.tile([128, 128], f32, tag="mm",
                                      name=f"tp{m}_{s4}")
                        nc.tensor.matmul(tp[:, :], ots[s4][:, :],
                                         ident[:, :], start=True, stop=True)
                        oT = otTp.tile([128, 128], bf16, tag="otT",
                                       name=f"otT{m}_{s4}")
                        nc.vector.tensor_copy(oT[:, :], tp[:, :])
                        state[s4] = oT
                    return t

                def mk_P(s4):
                    def t():
                        oT = state[s4]
                        for jc in range(2):
                            op = mmp.tile([128, 512], f32, tag="mm",
                                          name=f"op{m}_{s4}_{jc}")
                            nc.tensor.matmul(
                                op[:, :], oT[:, :],
                                wo[:, jc * 512:(jc + 1) * 512],
                                start=True, stop=True)
                            osb = osp.tile([128, 512], bf16, tag="outsb",
                                           name=f"osb{m}_{s4}_{jc}")
                            if tail and jc == 1:
                                nc.scalar.copy(osb[:, :], op[:, :])
                            else:
                                nc.vector.tensor_copy(osb[:, :], op[:, :])
                            r0 = q0 + s4 * 128
                            nc.sync.dma_start(
                                out=outp.ap()[r0:r0 + 128,
                                              jc * 512:(jc + 1) * 512],
                                in_=osb[:, :])
                        if s4 == 3:
                            combo_ot.pop(m)
                    return t

                def seq(*fs):
                    def t():
                        for f in fs:
                            f()
                    return t

                Ts = [mk_T(s) for s in range(4)]
                Ps = [mk_P(s) for s in range(4)]
                return [Ts[0], seq(Ps[0], Ts[1]), seq(Ps[1], Ts[2]),
                        seq(Ps[2], Ts[3]), Ps[3]]

            # ---- per-combo state ----
            combo_ot = {}    # m -> list of 4 onorm_tok tiles

            def emit_trans_out(m):
                # transpose (both heads at once) + outproj + out DMA for
                # combo m (emitted one combo later)
                b, qc = divmod(m, NQC)
                q0 = b * S + qc * 512
                ots = combo_ot.pop(m)
                oTs = []
                for s4 in range(4):
                    tp = mmp.tile([128, 128], f32, tag="mm",
                                  name=f"tp{m}_{s4}")
                    nc.tensor.matmul(tp[:, :], ots[s4][:, :], ident[:, :],
                                     start=True, stop=True)
                    oT = otTp.tile([128, 128], bf16, tag="otT",
                                   name=f"otT{m}_{s4}")
                    nc.vector.tensor_copy(oT[:, :], tp[:, :])
                    oTs.append(oT)
                for s4 in range(4):
                    oT = oTs[s4]
                    for jc in range(2):
                        op = mmp.tile([128, 512], f32, tag="mm",
                                      name=f"op{m}_{s4}_{jc}")
                        nc.tensor.matmul(
                            op[:, :], oT[:, :],
                            wo[:, jc * 512:(jc + 1) * 512],
                            start=True, stop=True)
                        osb = osp.tile([128, 512], bf16, tag="outsb",
                                       name=f"osb{m}_{s4}_{jc}")
                        nc.vector.tensor_copy(osb[:, :], op[:, :])
                        r0 = q0 + s4 * 128
                        nc.sync.dma_start(
                            out=outp.ap()[r0:r0 + 128,
                                          jc * 512:(jc + 1) * 512],
                            in_=osb[:, :])

            # ---- combo emission ----
            def emit_combo(m, tasks, vu_list=(), norm_prev=None):
                """tasks: micro-task list, one popped per ktile; vu_list:
                V-proj units (b, kt) to spread one-per-ktile; norm_prev:
                the previous combo's deferred norm closure (emitted after
                ktile 1's exp so its Ln/Exp never stall the scalar
                queue).  AV matmuls for ktile k are interleaved right
                after the scores of ktile k+2 (exp(k) is done by then
                thanks to the st double-buffer pacing), so the PE never
                piles a serial AV block between combos."""
                b, qc = divmod(m, NQC)
                q0 = b * S + qc * 512
                v4 = vtm[b].rearrange("p (h k c) -> p h k c", h=NH, k=NKT)
                tasks = list(tasks)
                pt = ptp.tile([128, NH * NKT * 512], bf16, tag="pt",
                              name=f"pt{m}")
                pt3 = pt.rearrange("p (h k q) -> p h k q", h=NH, k=NKT)

                # AV swapped: pt stationary [128k,128q], V|1 moving (N=65).
                # The 8 (h,s4) accumulation groups interleave within shared
                # PSUM banks, so hardware zero-on-start (2KB region
                # granularity) would wipe neighbors: the tile is memset
                # (by the preamble / previous norm chain) and accumulated
                # with start=False throughout.
                def emit_av(kti):
                    for h in range(NH):
                        for s4 in range(4):
                            nc.tensor.matmul(
                                av4[:, h, s4, :],
                                pt3[:, h, kti, s4 * 128:(s4 + 1) * 128],
                                v4[:, h, kti, :],
                                start=False, stop=(kti == NKT - 1),
                                skip_group_check=True)

                for kti in range(NKT):
                    k0 = b * S + kti * 128
                    st = stp.tile([128, 1024], f32, tag="st",
                                  name=f"st{m}_{kti}")
                    for h in range(NH):
                        hp = h * DH
                        nc.tensor.matmul(
                            st[:, h * 512:(h + 1) * 512],
                            kt[hp:hp + DH, k0:k0 + 128],
                            qt[hp:hp + DH, q0:q0 + 512],
                            start=True, stop=True)
                    nc.scalar.activation(
                        pt3[:, :, kti, :], st[:, :],
                        mybir.ActivationFunctionType.Exp, scale=SCALE)
                    if kti == 1 and norm_prev is not None:
                        norm_prev()
                    if kti < len(vu_list):
                        emit_VU(*vu_list[kti])
                    if kti >= 2:
                        emit_av(kti - 2)
                    if tasks:
                        tasks.pop(0)()
                emit_av(NKT - 2)
                emit_av(NKT - 1)
                for t in tasks:
                    t()

                def norm():
                    # denoms -> SBUF, recip via Ln/Exp on ScalarE, DVE
                    # mult, then re-zero the shared AV accumulator for the
                    # next combo (after all reads).
                    dn = dnp.tile([128, 2 * NH * 4], f32, tag="dn",
                                  name=f"dn{m}")
                    dn4 = dn.rearrange("p (g h s) -> p g h s", g=2, h=NH)
                    dn3 = dn4[:, 0, :, :]
                    rc3 = dn4[:, 1, :, :]
                    nc.vector.tensor_copy(dn3[:, :, :], av4[:, :, :, DH])
                    nc.scalar.activation(
                        rc3[:, :, :], dn3[:, :, :],
                        mybir.ActivationFunctionType.Ln)
                    nc.scalar.activation(
                        rc3[:, :, :], rc3[:, :, :],
                        mybir.ActivationFunctionType.Exp, scale=-1.0)
                    ots = []
                    for s4 in range(4):
                        ot = otp.tile([128, 128], bf16, tag="ot",
                                      name=f"ot{m}_{s4}")
                        for h in range(NH):
                            nc.vector.tensor_scalar(
                                ot[:, h * DH:(h + 1) * DH],
                                av4[:, h, s4, 0:DH],
                                rc3[:, h, s4:s4 + 1], None,
                                op0=mybir.AluOpType.mult)
                        ots.append(ot)
                    combo_ot[m] = ots
                    nc.vector.memset(av[:, :], 0.0)
                return norm

            # ---- schedule ----
            # PE warmup while the first x^T chunk DMA is in flight: ~16
            # dummy matmuls on the identity tile ramp the PE p-state so
            # the first projection chain runs at speed.
            for wi in range(40):
                wps = mmp.tile([128, 128], f32, tag="mm",
                               name=f"warm{wi}")
                nc.tensor.matmul(wps[:, :], ident[:, :], ident[:, :],
                                 start=True, stop=True)
            emit_KU(0, 0)
            emit_QU(0, 0)
            n0 = emit_combo(0,
                            KU_tasks(0, 1) + KU_tasks(0, 2)
                            + KU_tasks(0, 3) + QU_tasks(0, 1),
                            vu_list=[(0, k) for k in range(16)])
            n1 = emit_combo(1, QU_tasks(0, 2) + TO_tasks(0),
                            norm_prev=n0)
            n2 = emit_combo(2,
                            QU_tasks(0, 3) + TO_tasks(1) + KU_tasks(1, 0)
                            + [VU_task(1, 0), VU_task(1, 1)],
                            norm_prev=n1)
            n3 = emit_combo(3,
                            QU_tasks(1, 0) + TO_tasks(2) + KU_tasks(1, 1)
                            + KU_tasks(1, 2)
                            + [VU_task(1, k) for k in range(2, 6)],
                            norm_prev=n2)
            n4 = emit_combo(4,
                            QU_tasks(1, 1) + KU_tasks(1, 3)
                            + [VU_task(1, k) for k in range(6, 16)],
                            norm_prev=n3)
            n5 = emit_combo(5, QU_tasks(1, 2) + TO_tasks(3) + TO_tasks(4),
                            norm_prev=n4)
            n6 = emit_combo(6, QU_tasks(1, 3) + TO_tasks(5),
                            norm_prev=n5)
            n7 = emit_combo(7, [lambda: None, lambda: None] + TO_tasks(6),
                            norm_prev=n6)
            n7()
            for t in TO_tasks(7, tail=True):
                t()

    nc.compile()
    return nc


def _prep_inputs(x, Wq, bq, Wk, bk, Wv, bv, Wo, bo):
    bf16 = ml_dtypes.bfloat16
    xT = np.ascontiguousarray(
        np.asarray(x, dtype=np.float32).reshape(T, D).T).astype(bf16)
    in_maps = []
    for c in range(NCORES):
        cs = slice(c * 128, (c + 1) * 128)
        in_maps.append({
            "xT": xT,
            "wq": np.ascontiguousarray(Wq[:, cs]).astype(bf16),
            "wk": np.ascontiguousarray(Wk[:, cs]).astype(bf16),
            "wv": np.ascontiguousarray(Wv[:, cs]).astype(bf16),
            "wo": np.ascontiguousarray(Wo[cs, :]).astype(bf16),
            "bq": np.ascontiguousarray(bq[cs]).reshape(128, 1).astype(np.float32),
            "bk": np.ascontiguousarray(bk[cs]).reshape(128, 1).astype(np.float32),
        })
    return in_maps


def kernel(x, Wq, bq, Wk, bk, Wv, bv, Wo, bo, _trace=False, _results=None):
    from concourse.bass_utils import run_bass_kernel_spmd

    x = np.asarray(x); Wq = np.asarray(Wq); Wk = np.asarray(Wk)
    Wv = np.asarray(Wv); Wo = np.asarray(Wo)
    bq = np.asarray(bq); bk = np.asarray(bk); bv = np.asarray(bv)
    bo = np.asarray(bo)

    if "nc" not in _CACHE:
        _CACHE["nc"] = _build_nc()
    nc = _CACHE["nc"]

    in_maps = _prep_inputs(x, Wq, bq, Wk, bk, Wv, bv, Wo, bo)
    res = run_bass_kernel_spmd(
        nc, in_maps, core_ids=list(range(NCORES)), trace=_trace)
    if _results is not None:
        _results.append(res)

    acc = np.zeros((T, D), dtype=np.float32)
    for c in range(NCORES):
        acc += np.asarray(res.results[c]["outp"], dtype=np.float32)
    acc += bv.astype(np.float32) @ Wo.astype(np.float32) + bo.astype(np.float32)
    return acc.reshape(B, S, D)
